# revision 12
# baseline (speedup 1.0000x reference)
"""PatchMatch-style MatchingPropagator on 8 Trainium2 NeuronCores.

Full inputs in, full outputs out. Sharding: 8 independent units =
(direction in {forward, backward}) x (batch 0..3), one NeuronCore each.

Key layout decisions:
- The host re-packs each unit's correlation volume into "quad" records
  Q[n, y0, x0, 0:4] = corr[n, y0:y0+2, x0:x0+2] for anchors in [0,62]^2,
  so every bilinear sample is ONE contiguous 16-byte indirect-DMA fetch.
  Clamping floors to <=62 is numerically identical to the reference's
  corner clamping.
- Every DVE op on the critical path reads/writes contiguous (or at most
  3-dim strided) access patterns; measured on TRN2, deep strided/broadcast
  views cost 2-3x a contiguous op of the same size.
- Candidate coords live in CC = [x-cols | y-cols] so floor/clamp/index
  ops are single wide contiguous ops; the [x|y|s] accept blocks in CT are
  filled by copies hidden under the gather's DMA flight time.
- The score uses prebuilt interleaved weight tiles UW = [u w u w] and
  TW = [t t wy wy] per pixel (built off the critical path), so the score
  is 2 contiguous multiplies + 3 stride-4 adds, bit-exact against the
  reference's product/sum order: s = ((t1+t2)+t3)+t4.
- The initial score eval is folded into the first propagate's gather
  (candidates pre-rolled on the host): 7 gathers total.

Pixel layout on chip: pixel (i, j) -> partition 64*(j//32) + i, free j%32.
"""

import numpy as np

B, H, W = 4, 64, 64
R = 3.0
EPS = np.float32(0.01)
N_CORES = 8
PIX = H * W              # 4096 pixels per unit
AN = W - 1               # 63 anchors per axis in the quad layout
QROW = AN * 4            # 252 floats per anchor row
QMAP = AN * AN * 4       # 15876 floats per pixel quad map
M_RNE = float(1 << 23)

_CACHE = {}


# ----------------------------------------------------------------------------
# Device program (SPMD: identical on all 8 cores; data differs per core)
# ----------------------------------------------------------------------------

def _build_program():
    import concourse.bass as bass
    import concourse.mybir as mybir
    import concourse.tile as tile
    from concourse import bacc

    F32 = mybir.dt.float32
    I32 = mybir.dt.int32
    OP = mybir.AluOpType
    AF = mybir.ActivationFunctionType

    nc = bacc.Bacc(
        "TRN2",
        target_bir_lowering=False,
        debug=False,
        enable_asserts=False,
        num_devices=N_CORES,
    )

    corr = nc.dram_tensor("corr", [PIX * QMAP], F32, kind="ExternalInput")
    # state cols (32 each): [x, y, hx1, hy1, vx1, vy1, base, nx1, ny1,
    #                        nx2, ny2, nx3, ny3]
    state_in = nc.dram_tensor("state", [128, 13 * 32], F32,
                              kind="ExternalInput")
    out_xy = nc.dram_tensor("out_xy", [128, 64], F32,
                            kind="ExternalOutput")

    corr_flat = corr.ap().rearrange("(n one) -> n one", one=1)

    def b3(ap):  # [128,32] -> broadcast [128,3,32]
        return ap.rearrange("p (one f) -> p one f", one=1).to_broadcast(
            [128, 3, 32])

    with tile.TileContext(nc) as tc:
        with tc.tile_pool(name="main", bufs=1) as pool:
            ST = pool.tile([128, 13 * 32], F32, name="ST")
            nc.sync.dma_start(ST[0:64], state_in.ap()[0:64])
            nc.scalar.dma_start(ST[64:128], state_in.ap()[64:128])
            BASE = ST[:, 192:224]

            def noise_view(k):
                o = 224 + 64 * k
                return ST[:, o:o + 64]  # [nx|ny]

            # CT accept blocks of 96: [BEST | H | V], each [x|y|s]
            CT = pool.tile([128, 288], F32, name="CT")
            # candidate coords as [x y] pairs; separate tiles so the
            # v row-roll DMA never serializes against h-chain DVE writes
            CCH = pool.tile([128, 64], F32, name="CCH")
            CCV = pool.tile([128, 64], F32, name="CCV")
            G = pool.tile([128, 384], F32, name="G")
            UW = pool.tile([128, 384], F32, name="UW")   # [u w u w] per px
            TW = pool.tile([128, 384], F32, name="TW")   # [t t wy wy] per px
            WT = pool.tile([128, 192], F32, name="WT")   # [w | wy] per slot
            XI = pool.tile([128, 192], I32, name="XI")
            IF = pool.tile([128, 96], I32, name="IF")
            I = pool.tile([128, 96], I32, name="I")
            B1 = pool.tile([128, 384], F32, name="B1")
            B2 = pool.tile([128, 384], F32, name="B2")
            UPD = pool.tile([128, 96], I32, name="UPD")
            BASEI = pool.tile([128, 32], I32, name="BASEI")
            v0 = nc.vector
            v0.memset(I[:, 0:32], 0)
            nc.gpsimd.indirect_dma_start(
                out=G[:, 0:128],
                out_offset=None,
                in_=corr_flat,
                in_offset=bass.IndirectOffsetOnAxis(ap=I[:, 0:32], axis=0),
            )
            v0.tensor_copy(BASEI[:], ST[:, 192:224])

            v = nc.vector

            def eval_pre(cv, ne, off):
                """floor + clamp + quad indices for an eval slot of `ne`
                candidates whose [x y]-pair coords are the contiguous view
                cv.  Floors via truncating f32->i32 cast (coords >= 0),
                clamped <= 61+1 in int.  Slot regions start at 32-col
                block `off`."""
                n = 64 * ne
                m = 32 * ne
                x0 = XI[:, 2 * 32 * off:2 * 32 * off + n]
                v.tensor_scalar(x0, cv, float(AN - 1), None, OP.min)
                x2 = x0.rearrange("p (c s q) -> p c s q", c=ne, s=2)
                if3 = IF[:, 32 * off:32 * off + m].rearrange(
                    "p (e q) -> p e q", e=ne)
                i3 = I[:, 32 * off:32 * off + m].rearrange(
                    "p (e q) -> p e q", e=ne)
                baseb = (BASEI.rearrange("p (one f) -> p one f", one=1)
                         .to_broadcast([128, ne, 32]))
                v.scalar_tensor_tensor(if3, x2[:, :, 1], QROW, baseb,
                                       OP.mult, OP.add)
                v.scalar_tensor_tensor(i3, x2[:, :, 0], 4, if3,
                                       OP.mult, OP.add)

            def eval_gather(ne, off):
                nc.gpsimd.indirect_dma_start(
                    out=G[:, 128 * off:128 * (off + ne)],
                    out_offset=None,
                    in_=corr_flat,
                    in_offset=bass.IndirectOffsetOnAxis(
                        ap=I[:, 32 * off:32 * (off + ne)], axis=0),
                )

            def eval_weights(cv, ne, off):
                """hidden under gather flight: build UW = [u w u w] and
                TW = [t t wy wy] per pixel (all on DVE; no act tables)."""
                m = 32 * ne
                c2 = cv.rearrange("p (c s q) -> p c s q", c=ne, s=2)
                x2 = (XI[:, 64 * off:64 * off + 2 * m]
                      .rearrange("p (c s q) -> p c s q", c=ne, s=2))
                wcol = WT[:, 64 * off:64 * off + m]
                wycol = WT[:, 64 * off + m:64 * off + 2 * m]
                w = wcol.rearrange("p (e q) -> p e q", e=ne)
                wy = wycol.rearrange("p (e q) -> p e q", e=ne)
                v.tensor_tensor(w, c2[:, :, 0], x2[:, :, 0], OP.subtract)
                v.tensor_tensor(wy, c2[:, :, 1], x2[:, :, 1], OP.subtract)
                uwv = UW[:, 128 * off:128 * (off + ne)].rearrange(
                    "p (e d s) -> p e d s", e=m, d=2, s=2)
                twv = TW[:, 128 * off:128 * (off + ne)].rearrange(
                    "p (e s d) -> p e s d", e=m, s=2, d=2)
                wb = (wcol.rearrange("p (e one) -> p e one", one=1)
                      .to_broadcast([128, m, 2]))
                wyb = (wycol.rearrange("p (e one) -> p e one", one=1)
                       .to_broadcast([128, m, 2]))
                v.tensor_copy(uwv[:, :, :, 1], wb)
                v.tensor_copy(twv[:, :, 1, :], wyb)
                v.tensor_scalar(uwv[:, :, :, 0], wb, -1.0, 1.0,
                                OP.mult, OP.add)
                v.tensor_scalar(twv[:, :, 0, :], wyb, -1.0, 1.0,
                                OP.mult, OP.add)

            def eval_score(ne, off, sc_dst):
                """bilinear score; bit-exact term/sum order of the
                reference: t_k = (corner*u_or_w)*t_or_wy,
                s = ((t1+t2)+t3)+t4 via a sequential innermost reduce."""
                lo, hi = 128 * off, 128 * (off + ne)
                v.tensor_tensor(B1[:, lo:hi], G[:, lo:hi], UW[:, lo:hi],
                                OP.mult)
                v.tensor_tensor(B2[:, lo:hi], B1[:, lo:hi], TW[:, lo:hi],
                                OP.mult)
                b4 = B2[:, lo:hi].rearrange("p (e k) -> p e k", k=4)
                v.tensor_reduce(sc_dst, b4, mybir.AxisListType.X, OP.add)

            def accept(blk, last=False):
                """BEST = candidate block blk where its score is higher."""
                so = 96 * blk

                def b2(ap):
                    return (ap.rearrange("p (one f) -> p one f", one=1)
                            .to_broadcast([128, 2, 32]))

                v.tensor_tensor(UPD[:, 0:64].rearrange(
                    "p (c f) -> p c f", c=2), b2(CT[:, so + 64:so + 96]),
                    b2(CT[:, 64:96]), OP.is_gt)
                v.copy_predicated(CT[:, 0:64], UPD[:, 0:64],
                                  CT[:, so:so + 64])
                if not last:
                    v.copy_predicated(CT[:, 64:96], UPD[:, 0:32],
                                      CT[:, so + 64:so + 96])

            def sc_block(blk, nb=1):
                """CT score-column view [128, nb, 32] from block blk."""
                return (CT[:].rearrange("p (b f) -> p b f", b=3)
                        [:, blk:blk + nb, 64:96])

            # ---- round 1: initial eval + propagate(1,1); candidate coords
            # pre-rolled on the host.  Split into a (best,h) chain and a v
            # chain so the second gather's descriptor gen overlaps the
            # first's flight.
            eval_pre(ST[:, 0:192], 3, 0)
            eval_gather(3, 0)
            eval_weights(ST[:, 0:192], 3, 0)
            v.tensor_copy(CT[:, 0:64], ST[:, 0:64])
            v.tensor_copy(CT[:, 96:160], ST[:, 64:128])
            v.tensor_copy(CT[:, 192:256], ST[:, 128:192])
            eval_score(3, 0, sc_block(0, 3))
            accept(1)
            accept(2)

            def propagate(dx, dy, last=False):
                # cand_v coords: row-roll of BEST [x|y] by dy via 2 fused-AP
                # DMAs (bulk + wrap) on the two HWDGE issuers; issued first
                # so the DMA latency overlaps the whole h-chain prep
                dvv = CCV[:].rearrange("(b i) f -> b i f", b=2)
                svv = CT[:, 0:64].rearrange("(b i) f -> b i f", b=2)
                if dy == 1:
                    nc.sync.dma_start(dvv[:, 1:64], svv[:, 0:63])
                    nc.scalar.dma_start(dvv[:, 0:1], svv[:, 63:64])
                else:
                    nc.sync.dma_start(dvv[:, 0:63], svv[:, 1:64])
                    nc.scalar.dma_start(dvv[:, 63:64], svv[:, 0:1])

                # ---- h chain: col-roll by dx (DVE), clamp, idx; its DVE
                # work and weight prep hide the v row-roll DMA latency
                dh = CCH[:].rearrange("p (c f) -> p c f", c=2)
                sh = CT[:, 0:64].rearrange("p (c f) -> p c f", c=2)
                if dx == 1:
                    v.tensor_copy(dh[:, :, 1:32], sh[:, :, 0:31])
                    v.tensor_copy(dh[64:128, :, 0:1], sh[0:64, :, 31:32])
                    v.tensor_copy(dh[0:64, :, 0:1], sh[64:128, :, 31:32])
                    v.tensor_scalar(CCH[:, 0:32], CCH[:, 0:32], 1.0,
                                    float(W - 1), OP.add, OP.min)
                else:
                    v.tensor_copy(dh[:, :, 0:31], sh[:, :, 1:32])
                    v.tensor_copy(dh[0:64, :, 31:32], sh[64:128, :, 0:1])
                    v.tensor_copy(dh[64:128, :, 31:32], sh[0:64, :, 0:1])
                    v.tensor_scalar(CCH[:, 0:32], CCH[:, 0:32], -1.0, 0.0,
                                    OP.add, OP.max)
                eval_pre(CCH[:], 1, 0)
                eval_weights(CCH[:], 1, 0)
                v.tensor_copy(CT[:, 96:160], CCH[:])

                # ---- v chain (waits the roll DMA), then one fused gather
                if dy == 1:
                    v.tensor_scalar(CCV[:, 32:64], CCV[:, 32:64], 1.0,
                                    float(H - 1), OP.add, OP.min)
                else:
                    v.tensor_scalar(CCV[:, 32:64], CCV[:, 32:64], -1.0,
                                    0.0, OP.add, OP.max)
                eval_pre(CCV[:], 1, 1)
                eval_gather(2, 0)

                # hidden work under the gather flight
                eval_weights(CCV[:], 1, 1)
                v.tensor_copy(CT[:, 192:256], CCV[:])
                eval_score(2, 0, sc_block(1, 2))
                accept(1)
                accept(2, last=last)

            def random_search(k):
                v.tensor_tensor(CCH[:], CT[:, 0:64], noise_view(k),
                                OP.add)
                v.tensor_scalar(CCH[:], CCH[:], 0.0,
                                float(W - 1), OP.max, OP.min)
                eval_pre(CCH[:], 1, 0)
                eval_gather(1, 0)
                eval_weights(CCH[:], 1, 0)
                v.tensor_copy(CT[:, 96:160], CCH[:])
                eval_score(1, 0, sc_block(1))
                accept(1)

            random_search(0)
            propagate(-1, -1)
            random_search(1)
            propagate(-1, 1)
            random_search(2)
            propagate(1, -1, last=True)

            nc.sync.dma_start(out_xy.ap(), CT[:, 0:64])

    nc.compile()
    return nc


def _get_program():
    if "nc" not in _CACHE:
        _CACHE["nc"] = _build_program()
    return _CACHE["nc"]


# ----------------------------------------------------------------------------
# Host-side helpers
# ----------------------------------------------------------------------------

def _to_layout(v):
    """[64(i), 64(j)] -> [128, 32]; partition = 64*(j//32)+i, free = j%32."""
    return np.ascontiguousarray(
        v.reshape(64, 2, 32).transpose(1, 0, 2).reshape(128, 32))


def _from_layout(a):
    """[128, 32] -> [64(i), 64(j)]."""
    return a.reshape(2, 64, 32).transpose(1, 0, 2).reshape(64, 64)


def _noise_arrays():
    """Mirror the reference's jax.random usage exactly, in-process, so the
    values match the grader's reference no matter which jax backend/PRNG
    the process defaults to."""
    import jax
    import jax.numpy as jnp

    key = jax.random.key(42)
    kf, kb = jax.random.split(key)
    out = []
    for kdir in (kf, kb):
        ks = jax.random.split(kdir, 3)
        out.append([np.asarray(R * jax.random.normal(k, (B, H, W, 2),
                                                     jnp.float32))
                    for k in ks])
    return out  # [dir][step] -> [B,H,W,2] float32


def _quad_pack(corr_u):
    """[4096, 64, 64] -> flat quad records [4096*63*63*4] f32."""
    sw = np.lib.stride_tricks.sliding_window_view(corr_u, (2, 2),
                                                  axis=(1, 2))
    # sw: [4096, 63, 63, 2, 2]
    return np.ascontiguousarray(sw).reshape(-1)


def _make_state(x_plane, y_plane, noise_steps, b):
    """Build the [128, 13*32] per-core state tensor (partition-major)."""
    x = x_plane.astype(np.float32)
    y = y_plane.astype(np.float32)
    one = np.float32(1.0)
    # first propagate is (dx, dy) = (1, 1); host pre-rolls the candidates
    hx = np.clip(np.roll(x, 1, axis=1) + one, np.float32(0.0),
                 np.float32(W - 1))
    hy = np.roll(y, 1, axis=1)
    vx = np.roll(x, 1, axis=0)
    vy = np.clip(np.roll(y, 1, axis=0) + one, np.float32(0.0),
                 np.float32(H - 1))
    base = ((np.arange(64, dtype=np.int64)[:, None] * 64
             + np.arange(64, dtype=np.int64)[None, :]) * QMAP)
    rows = [
        _to_layout(x), _to_layout(y),
        _to_layout(hx), _to_layout(hy),
        _to_layout(vx), _to_layout(vy),
        _to_layout(base.astype(np.float32)),
    ]
    for step in range(3):
        nz = noise_steps[step][b]  # [H,W,2]
        rows.append(_to_layout(np.ascontiguousarray(nz[:, :, 0])))
        rows.append(_to_layout(np.ascontiguousarray(nz[:, :, 1])))
    return np.concatenate(rows, axis=1).astype(np.float32)


def _bilinear_map_np(img, coords):
    """numpy mirror of reference._bilinear_map (fp32, same op order).
    img [B,H,W,C], coords [B,H,W,2] -> [B,H,W,C]"""
    Bn, Hn, Wn, C = img.shape
    out = np.empty_like(img)
    one = np.float32(1.0)
    for b in range(Bn):
        x = coords[b, :, :, 0].reshape(-1)
        y = coords[b, :, :, 1].reshape(-1)
        x0 = np.floor(x)
        y0 = np.floor(y)
        wx = (x - x0)[:, None]
        wy = (y - y0)[:, None]
        x0i = np.clip(x0.astype(np.int32), 0, Wn - 1)
        x1i = np.clip(x0i + 1, 0, Wn - 1)
        y0i = np.clip(y0.astype(np.int32), 0, Hn - 1)
        y1i = np.clip(y0i + 1, 0, Hn - 1)
        im = img[b]
        v00 = im[y0i, x0i]
        v01 = im[y0i, x1i]
        v10 = im[y1i, x0i]
        v11 = im[y1i, x1i]
        o = (v00 * (one - wx) * (one - wy) + v01 * wx * (one - wy)
             + v10 * (one - wx) * wy + v11 * wx * wy)
        out[b] = o.reshape(Hn, Wn, C)
    return out


def _run_device(in_maps, trace=False):
    from concourse import bass_utils

    nc = _get_program()
    res = bass_utils.run_bass_kernel_spmd(
        nc, in_maps, core_ids=list(range(N_CORES)), trace=trace)
    return res


def kernel(matching_f, matching_b, corr_map, _trace=False, _results_hook=None):
    matching_f = np.asarray(matching_f)
    matching_b = np.asarray(matching_b)
    corr_map = np.asarray(corr_map)

    noise = _noise_arrays()  # [dir][step][B,H,W,2]

    in_maps = []
    for b in range(B):  # forward units, cores 0..3
        corr_u = np.ascontiguousarray(corr_map[b]).reshape(PIX, H, W)
        in_maps.append({
            "corr": _quad_pack(corr_u),
            "state": _make_state(matching_f[b, 0], matching_f[b, 1],
                                 noise[0], b),
        })
    for b in range(B):  # backward units, cores 4..7
        corr_t = np.ascontiguousarray(
            corr_map[b].transpose(2, 3, 0, 1)).reshape(PIX, H, W)
        in_maps.append({
            "corr": _quad_pack(corr_t),
            "state": _make_state(matching_b[b, 0], matching_b[b, 1],
                                 noise[1], b),
        })

    res = _run_device(in_maps, trace=_trace)
    if _results_hook is not None:
        _results_hook(res)

    res_f = np.empty((B, H, W, 2), np.float32)
    res_b = np.empty((B, H, W, 2), np.float32)
    for b in range(B):
        of = res.results[b]["out_xy"]
        ob = res.results[4 + b]["out_xy"]
        res_f[b, :, :, 0] = _from_layout(of[:, 0:32])
        res_f[b, :, :, 1] = _from_layout(of[:, 32:64])
        res_b[b, :, :, 0] = _from_layout(ob[:, 0:32])
        res_b[b, :, :, 1] = _from_layout(ob[:, 32:64])

    # forward-backward consistency (host; mirrors reference in fp32)
    counter = _bilinear_map_np(res_b, res_f)
    diff = np.max(np.abs(res_f - counter), axis=-1)
    invalid = (diff > EPS)[..., None]
    mf_t = matching_f.transpose(0, 2, 3, 1)  # [B,H,W,2]
    out = np.where(invalid, mf_t, res_f)
    return np.ascontiguousarray(out.transpose(0, 3, 1, 2)).astype(np.float32)


# revision 13
# speedup vs baseline: 1.0459x; 1.0459x over previous
"""PatchMatch-style MatchingPropagator on 8 Trainium2 NeuronCores.

Full inputs in, full outputs out. Sharding: 8 independent units =
(direction in {forward, backward}) x (batch 0..3), one NeuronCore each.

Key layout decisions:
- The host re-packs each unit's correlation volume into "quad" records
  Q[n, y0, x0, 0:4] = corr[n, y0:y0+2, x0:x0+2] for anchors in [0,62]^2,
  so every bilinear sample is ONE contiguous 16-byte indirect-DMA fetch.
  Clamping floors to <=62 is numerically identical to the reference's
  corner clamping.
- Every DVE op on the critical path reads/writes contiguous (or at most
  3-dim strided) access patterns; measured on TRN2, deep strided/broadcast
  views cost 2-3x a contiguous op of the same size.
- Candidate coords live in CC = [x-cols | y-cols] so floor/clamp/index
  ops are single wide contiguous ops; the [x|y|s] accept blocks in CT are
  filled by copies hidden under the gather's DMA flight time.
- The score uses prebuilt interleaved weight tiles UW = [u w u w] and
  TW = [t t wy wy] per pixel (built off the critical path), so the score
  is 2 contiguous multiplies + 3 stride-4 adds, bit-exact against the
  reference's product/sum order: s = ((t1+t2)+t3)+t4.
- The initial score eval is folded into the first propagate's gather
  (candidates pre-rolled on the host): 7 gathers total.

Pixel layout on chip: pixel (i, j) -> partition 64*(j//32) + i, free j%32.
"""

import numpy as np

B, H, W = 4, 64, 64
R = 3.0
EPS = np.float32(0.01)
N_CORES = 8
PIX = H * W              # 4096 pixels per unit
AN = W - 1               # 63 anchors per axis in the quad layout
QROW = AN * 4            # 252 floats per anchor row
QMAP = AN * AN * 4       # 15876 floats per pixel quad map
M_RNE = float(1 << 23)

_CACHE = {}


# ----------------------------------------------------------------------------
# Device program (SPMD: identical on all 8 cores; data differs per core)
# ----------------------------------------------------------------------------

def _build_program():
    import concourse.bass as bass
    import concourse.mybir as mybir
    import concourse.tile as tile
    from concourse import bacc

    F32 = mybir.dt.float32
    I32 = mybir.dt.int32
    OP = mybir.AluOpType
    AF = mybir.ActivationFunctionType

    nc = bacc.Bacc(
        "TRN2",
        target_bir_lowering=False,
        debug=False,
        enable_asserts=False,
        num_devices=N_CORES,
    )

    corr = nc.dram_tensor("corr", [PIX * QMAP], F32, kind="ExternalInput")
    # state cols (32 each): [x, y, hx1, hy1, vx1, vy1, base, nx1, ny1,
    #                        nx2, ny2, nx3, ny3]
    state_in = nc.dram_tensor("state", [128, 13 * 32], F32,
                              kind="ExternalInput")
    out_xy = nc.dram_tensor("out_xy", [128, 64], F32,
                            kind="ExternalOutput")

    corr_flat = corr.ap().rearrange("(n one) -> n one", one=1)

    def b3(ap):  # [128,32] -> broadcast [128,3,32]
        return ap.rearrange("p (one f) -> p one f", one=1).to_broadcast(
            [128, 3, 32])

    with tile.TileContext(nc) as tc:
        with tc.tile_pool(name="main", bufs=1) as pool:
            ST = pool.tile([128, 13 * 32], F32, name="ST")
            nc.sync.dma_start(ST[0:64], state_in.ap()[0:64])
            nc.scalar.dma_start(ST[64:128], state_in.ap()[64:128])
            BASE = ST[:, 192:224]

            def noise_view(k):
                o = 224 + 64 * k
                return ST[:, o:o + 64]  # [nx|ny]

            # CT accept blocks of 96: [BEST | H | V], each [x|y|s]
            CT = pool.tile([128, 288], F32, name="CT")
            # candidate coords as [x y] pairs; separate tiles so the
            # v row-roll DMA never serializes against h-chain DVE writes
            CCH = pool.tile([128, 64], F32, name="CCH")
            CCV = pool.tile([128, 64], F32, name="CCV")
            G = pool.tile([128, 384], F32, name="G")
            UW = pool.tile([128, 384], F32, name="UW")   # [u w u w] per px
            TW = pool.tile([128, 384], F32, name="TW")   # [t t wy wy] per px
            WT = pool.tile([128, 192], F32, name="WT")   # [w | wy] per slot
            XI = pool.tile([128, 192], I32, name="XI")
            IF = pool.tile([128, 96], I32, name="IF")
            I = pool.tile([128, 96], I32, name="I")
            B1 = pool.tile([128, 384], F32, name="B1")
            B2 = pool.tile([128, 384], F32, name="B2")
            UPD = pool.tile([128, 128], I32, name="UPD")
            RCS = pool.tile([128, 192], F32, name="RCS")  # 3-variant RC
            XIS = pool.tile([128, 192], I32, name="XIS")
            ISF = pool.tile([128, 96], I32, name="ISF")
            IS = pool.tile([128, 96], I32, name="IS")
            BASEI = pool.tile([128, 32], I32, name="BASEI")
            v0 = nc.vector
            v0.memset(I[:, 0:32], 0)
            nc.gpsimd.indirect_dma_start(
                out=G[:, 0:128],
                out_offset=None,
                in_=corr_flat,
                in_offset=bass.IndirectOffsetOnAxis(ap=I[:, 0:32], axis=0),
            )
            v0.tensor_copy(BASEI[:], ST[:, 192:224])

            v = nc.vector

            def eval_pre(cv, ne, off):
                """floor + clamp + quad indices for an eval slot of `ne`
                candidates whose [x y]-pair coords are the contiguous view
                cv.  Floors via truncating f32->i32 cast (coords >= 0),
                clamped <= 61+1 in int.  Slot regions start at 32-col
                block `off`."""
                n = 64 * ne
                m = 32 * ne
                x0 = XI[:, 2 * 32 * off:2 * 32 * off + n]
                v.tensor_scalar(x0, cv, float(AN - 1), None, OP.min)
                x2 = x0.rearrange("p (c s q) -> p c s q", c=ne, s=2)
                if3 = IF[:, 32 * off:32 * off + m].rearrange(
                    "p (e q) -> p e q", e=ne)
                i3 = I[:, 32 * off:32 * off + m].rearrange(
                    "p (e q) -> p e q", e=ne)
                baseb = (BASEI.rearrange("p (one f) -> p one f", one=1)
                         .to_broadcast([128, ne, 32]))
                v.scalar_tensor_tensor(if3, x2[:, :, 1], QROW, baseb,
                                       OP.mult, OP.add)
                v.scalar_tensor_tensor(i3, x2[:, :, 0], 4, if3,
                                       OP.mult, OP.add)

            def eval_gather(ne, off):
                nc.gpsimd.indirect_dma_start(
                    out=G[:, 128 * off:128 * (off + ne)],
                    out_offset=None,
                    in_=corr_flat,
                    in_offset=bass.IndirectOffsetOnAxis(
                        ap=I[:, 32 * off:32 * (off + ne)], axis=0),
                )

            def eval_weights(cv, ne, off):
                """hidden under gather flight: build UW = [u w u w] and
                TW = [t t wy wy] per pixel (all on DVE; no act tables)."""
                m = 32 * ne
                c2 = cv.rearrange("p (c s q) -> p c s q", c=ne, s=2)
                x2 = (XI[:, 64 * off:64 * off + 2 * m]
                      .rearrange("p (c s q) -> p c s q", c=ne, s=2))
                wcol = WT[:, 64 * off:64 * off + m]
                wycol = WT[:, 64 * off + m:64 * off + 2 * m]
                w = wcol.rearrange("p (e q) -> p e q", e=ne)
                wy = wycol.rearrange("p (e q) -> p e q", e=ne)
                v.tensor_tensor(w, c2[:, :, 0], x2[:, :, 0], OP.subtract)
                v.tensor_tensor(wy, c2[:, :, 1], x2[:, :, 1], OP.subtract)
                uwv = UW[:, 128 * off:128 * (off + ne)].rearrange(
                    "p (e d s) -> p e d s", e=m, d=2, s=2)
                twv = TW[:, 128 * off:128 * (off + ne)].rearrange(
                    "p (e s d) -> p e s d", e=m, s=2, d=2)
                wb = (wcol.rearrange("p (e one) -> p e one", one=1)
                      .to_broadcast([128, m, 2]))
                wyb = (wycol.rearrange("p (e one) -> p e one", one=1)
                       .to_broadcast([128, m, 2]))
                v.tensor_copy(uwv[:, :, :, 1], wb)
                v.tensor_copy(twv[:, :, 1, :], wyb)
                v.tensor_scalar(uwv[:, :, :, 0], wb, -1.0, 1.0,
                                OP.mult, OP.add)
                v.tensor_scalar(twv[:, :, 0, :], wyb, -1.0, 1.0,
                                OP.mult, OP.add)

            def eval_score(ne, off, sc_dst):
                """bilinear score; bit-exact term/sum order of the
                reference: t_k = (corner*u_or_w)*t_or_wy,
                s = ((t1+t2)+t3)+t4 via a sequential innermost reduce."""
                lo, hi = 128 * off, 128 * (off + ne)
                v.tensor_tensor(B1[:, lo:hi], G[:, lo:hi], UW[:, lo:hi],
                                OP.mult)
                v.tensor_tensor(B2[:, lo:hi], B1[:, lo:hi], TW[:, lo:hi],
                                OP.mult)
                b4 = B2[:, lo:hi].rearrange("p (e k) -> p e k", k=4)
                v.tensor_reduce(sc_dst, b4, mybir.AxisListType.X, OP.add)

            def accept(blk, last=False):
                """BEST = candidate block blk where its score is higher.
                The decision mask is kept in UPD slot blk-1 so a following
                speculative random-search can select by it."""
                so = 96 * blk
                mo = 64 * (blk - 1)

                def b2(ap):
                    return (ap.rearrange("p (one f) -> p one f", one=1)
                            .to_broadcast([128, 2, 32]))

                v.tensor_tensor(UPD[:, mo:mo + 64].rearrange(
                    "p (c f) -> p c f", c=2), b2(CT[:, so + 64:so + 96]),
                    b2(CT[:, 64:96]), OP.is_gt)
                v.copy_predicated(CT[:, 0:64], UPD[:, mo:mo + 64],
                                  CT[:, so:so + 64])
                if not last:
                    v.copy_predicated(CT[:, 64:96], UPD[:, mo:mo + 32],
                                      CT[:, so + 64:so + 96])

            def spec_rs(k):
                """Speculative random-search index precompute, hidden under
                the current gather's flight: candidate coords + quad
                indices for each possible accept outcome (B, H, V).
                The base variant lands directly in I/CCH; the accepts'
                masks later select the H/V variants."""
                cv3 = (CT[:].rearrange("p (b f) -> p b f", b=3)[:, :, 0:64])
                nzb = (noise_view(k)
                       .rearrange("p (one f) -> p one f", one=1)
                       .to_broadcast([128, 3, 64]))
                rc3 = RCS[:].rearrange("p (c f) -> p c f", c=3)
                v.tensor_tensor(rc3, cv3, nzb, OP.add)
                v.tensor_scalar(RCS[:], RCS[:], 0.0, float(W - 1),
                                OP.max, OP.min)
                v.tensor_scalar(XIS[:], RCS[:], float(AN - 1), None, OP.min)
                x2 = XIS[:].rearrange("p (c s q) -> p c s q", c=3, s=2)
                if3 = ISF[:].rearrange("p (e q) -> p e q", e=3)
                i3 = IS[:].rearrange("p (e q) -> p e q", e=3)
                baseb = (BASEI.rearrange("p (one f) -> p one f", one=1)
                         .to_broadcast([128, 3, 32]))
                v.scalar_tensor_tensor(if3, x2[:, :, 1], QROW, baseb,
                                       OP.mult, OP.add)
                v.scalar_tensor_tensor(i3, x2[:, :, 0], 4, if3,
                                       OP.mult, OP.add)
                v.tensor_copy(I[:, 0:32], IS[:, 0:32])
                v.tensor_copy(CCH[:], RCS[:, 0:64])

            def sc_block(blk, nb=1):
                """CT score-column view [128, nb, 32] from block blk."""
                return (CT[:].rearrange("p (b f) -> p b f", b=3)
                        [:, blk:blk + nb, 64:96])

            # ---- round 1: initial eval + propagate(1,1); candidate coords
            # pre-rolled on the host.  Split into a (best,h) chain and a v
            # chain so the second gather's descriptor gen overlaps the
            # first's flight.
            eval_pre(ST[:, 0:192], 3, 0)
            eval_gather(3, 0)
            eval_weights(ST[:, 0:192], 3, 0)
            v.tensor_copy(CT[:, 0:64], ST[:, 0:64])
            v.tensor_copy(CT[:, 96:160], ST[:, 64:128])
            v.tensor_copy(CT[:, 192:256], ST[:, 128:192])
            spec_rs(0)
            eval_score(3, 0, sc_block(0, 3))
            accept(1)
            accept(2)

            def propagate(dx, dy, spec_k=None, last=False):
                # cand_v coords: row-roll of BEST [x|y] by dy via 2 fused-AP
                # DMAs (bulk + wrap) on the two HWDGE issuers; issued first
                # so the DMA latency overlaps the whole h-chain prep
                dvv = CCV[:].rearrange("(b i) f -> b i f", b=2)
                svv = CT[:, 0:64].rearrange("(b i) f -> b i f", b=2)
                if dy == 1:
                    nc.sync.dma_start(dvv[:, 1:64], svv[:, 0:63])
                    nc.scalar.dma_start(dvv[:, 0:1], svv[:, 63:64])
                else:
                    nc.sync.dma_start(dvv[:, 0:63], svv[:, 1:64])
                    nc.scalar.dma_start(dvv[:, 63:64], svv[:, 0:1])

                # ---- h chain: col-roll by dx (DVE), clamp, idx; its DVE
                # work and weight prep hide the v row-roll DMA latency
                dh = CCH[:].rearrange("p (c f) -> p c f", c=2)
                sh = CT[:, 0:64].rearrange("p (c f) -> p c f", c=2)
                if dx == 1:
                    v.tensor_copy(dh[:, :, 1:32], sh[:, :, 0:31])
                    v.tensor_copy(dh[64:128, :, 0:1], sh[0:64, :, 31:32])
                    v.tensor_copy(dh[0:64, :, 0:1], sh[64:128, :, 31:32])
                    v.tensor_scalar(CCH[:, 0:32], CCH[:, 0:32], 1.0,
                                    float(W - 1), OP.add, OP.min)
                else:
                    v.tensor_copy(dh[:, :, 0:31], sh[:, :, 1:32])
                    v.tensor_copy(dh[0:64, :, 31:32], sh[64:128, :, 0:1])
                    v.tensor_copy(dh[64:128, :, 31:32], sh[0:64, :, 0:1])
                    v.tensor_scalar(CCH[:, 0:32], CCH[:, 0:32], -1.0, 0.0,
                                    OP.add, OP.max)
                eval_pre(CCH[:], 1, 0)
                eval_weights(CCH[:], 1, 0)
                v.tensor_copy(CT[:, 96:160], CCH[:])

                # ---- v chain (waits the roll DMA), then one fused gather
                if dy == 1:
                    v.tensor_scalar(CCV[:, 32:64], CCV[:, 32:64], 1.0,
                                    float(H - 1), OP.add, OP.min)
                else:
                    v.tensor_scalar(CCV[:, 32:64], CCV[:, 32:64], -1.0,
                                    0.0, OP.add, OP.max)
                eval_pre(CCV[:], 1, 1)
                eval_gather(2, 0)

                # hidden work under the gather flight
                eval_weights(CCV[:], 1, 1)
                v.tensor_copy(CT[:, 192:256], CCV[:])
                if spec_k is not None:
                    spec_rs(spec_k)
                eval_score(2, 0, sc_block(1, 2))
                accept(1)
                accept(2, last=last)

            def random_search(k):
                # indices were speculatively built for all 3 variants;
                # select by the propagate accept masks and fire the gather
                v.copy_predicated(I[:, 0:32], UPD[:, 0:32], IS[:, 32:64])
                v.copy_predicated(I[:, 0:32], UPD[:, 64:96], IS[:, 64:96])
                eval_gather(1, 0)
                # hidden under the flight: resolve coords the same way,
                # then weights from a fresh floor cast
                v.copy_predicated(CCH[:], UPD[:, 0:64], RCS[:, 64:128])
                v.copy_predicated(CCH[:], UPD[:, 64:128], RCS[:, 128:192])
                v.tensor_scalar(XI[:, 0:64], CCH[:], float(AN - 1),
                                None, OP.min)
                eval_weights(CCH[:], 1, 0)
                v.tensor_copy(CT[:, 96:160], CCH[:])
                eval_score(1, 0, sc_block(1))
                accept(1)

            random_search(0)
            propagate(-1, -1, spec_k=1)
            random_search(1)
            propagate(-1, 1, spec_k=2)
            random_search(2)
            propagate(1, -1, last=True)

            nc.sync.dma_start(out_xy.ap(), CT[:, 0:64])

    nc.compile()
    return nc


def _get_program():
    if "nc" not in _CACHE:
        _CACHE["nc"] = _build_program()
    return _CACHE["nc"]


# ----------------------------------------------------------------------------
# Host-side helpers
# ----------------------------------------------------------------------------

def _to_layout(v):
    """[64(i), 64(j)] -> [128, 32]; partition = 64*(j//32)+i, free = j%32."""
    return np.ascontiguousarray(
        v.reshape(64, 2, 32).transpose(1, 0, 2).reshape(128, 32))


def _from_layout(a):
    """[128, 32] -> [64(i), 64(j)]."""
    return a.reshape(2, 64, 32).transpose(1, 0, 2).reshape(64, 64)


def _noise_arrays():
    """Mirror the reference's jax.random usage exactly, in-process, so the
    values match the grader's reference no matter which jax backend/PRNG
    the process defaults to."""
    import jax
    import jax.numpy as jnp

    key = jax.random.key(42)
    kf, kb = jax.random.split(key)
    out = []
    for kdir in (kf, kb):
        ks = jax.random.split(kdir, 3)
        out.append([np.asarray(R * jax.random.normal(k, (B, H, W, 2),
                                                     jnp.float32))
                    for k in ks])
    return out  # [dir][step] -> [B,H,W,2] float32


def _quad_pack(corr_u):
    """[4096, 64, 64] -> flat quad records [4096*63*63*4] f32."""
    sw = np.lib.stride_tricks.sliding_window_view(corr_u, (2, 2),
                                                  axis=(1, 2))
    # sw: [4096, 63, 63, 2, 2]
    return np.ascontiguousarray(sw).reshape(-1)


def _make_state(x_plane, y_plane, noise_steps, b):
    """Build the [128, 13*32] per-core state tensor (partition-major)."""
    x = x_plane.astype(np.float32)
    y = y_plane.astype(np.float32)
    one = np.float32(1.0)
    # first propagate is (dx, dy) = (1, 1); host pre-rolls the candidates
    hx = np.clip(np.roll(x, 1, axis=1) + one, np.float32(0.0),
                 np.float32(W - 1))
    hy = np.roll(y, 1, axis=1)
    vx = np.roll(x, 1, axis=0)
    vy = np.clip(np.roll(y, 1, axis=0) + one, np.float32(0.0),
                 np.float32(H - 1))
    base = ((np.arange(64, dtype=np.int64)[:, None] * 64
             + np.arange(64, dtype=np.int64)[None, :]) * QMAP)
    rows = [
        _to_layout(x), _to_layout(y),
        _to_layout(hx), _to_layout(hy),
        _to_layout(vx), _to_layout(vy),
        _to_layout(base.astype(np.float32)),
    ]
    for step in range(3):
        nz = noise_steps[step][b]  # [H,W,2]
        rows.append(_to_layout(np.ascontiguousarray(nz[:, :, 0])))
        rows.append(_to_layout(np.ascontiguousarray(nz[:, :, 1])))
    return np.concatenate(rows, axis=1).astype(np.float32)


def _bilinear_map_np(img, coords):
    """numpy mirror of reference._bilinear_map (fp32, same op order).
    img [B,H,W,C], coords [B,H,W,2] -> [B,H,W,C]"""
    Bn, Hn, Wn, C = img.shape
    out = np.empty_like(img)
    one = np.float32(1.0)
    for b in range(Bn):
        x = coords[b, :, :, 0].reshape(-1)
        y = coords[b, :, :, 1].reshape(-1)
        x0 = np.floor(x)
        y0 = np.floor(y)
        wx = (x - x0)[:, None]
        wy = (y - y0)[:, None]
        x0i = np.clip(x0.astype(np.int32), 0, Wn - 1)
        x1i = np.clip(x0i + 1, 0, Wn - 1)
        y0i = np.clip(y0.astype(np.int32), 0, Hn - 1)
        y1i = np.clip(y0i + 1, 0, Hn - 1)
        im = img[b]
        v00 = im[y0i, x0i]
        v01 = im[y0i, x1i]
        v10 = im[y1i, x0i]
        v11 = im[y1i, x1i]
        o = (v00 * (one - wx) * (one - wy) + v01 * wx * (one - wy)
             + v10 * (one - wx) * wy + v11 * wx * wy)
        out[b] = o.reshape(Hn, Wn, C)
    return out


def _run_device(in_maps, trace=False):
    from concourse import bass_utils

    nc = _get_program()
    res = bass_utils.run_bass_kernel_spmd(
        nc, in_maps, core_ids=list(range(N_CORES)), trace=trace)
    return res


def kernel(matching_f, matching_b, corr_map, _trace=False, _results_hook=None):
    matching_f = np.asarray(matching_f)
    matching_b = np.asarray(matching_b)
    corr_map = np.asarray(corr_map)

    noise = _noise_arrays()  # [dir][step][B,H,W,2]

    in_maps = []
    for b in range(B):  # forward units, cores 0..3
        corr_u = np.ascontiguousarray(corr_map[b]).reshape(PIX, H, W)
        in_maps.append({
            "corr": _quad_pack(corr_u),
            "state": _make_state(matching_f[b, 0], matching_f[b, 1],
                                 noise[0], b),
        })
    for b in range(B):  # backward units, cores 4..7
        corr_t = np.ascontiguousarray(
            corr_map[b].transpose(2, 3, 0, 1)).reshape(PIX, H, W)
        in_maps.append({
            "corr": _quad_pack(corr_t),
            "state": _make_state(matching_b[b, 0], matching_b[b, 1],
                                 noise[1], b),
        })

    res = _run_device(in_maps, trace=_trace)
    if _results_hook is not None:
        _results_hook(res)

    res_f = np.empty((B, H, W, 2), np.float32)
    res_b = np.empty((B, H, W, 2), np.float32)
    for b in range(B):
        of = res.results[b]["out_xy"]
        ob = res.results[4 + b]["out_xy"]
        res_f[b, :, :, 0] = _from_layout(of[:, 0:32])
        res_f[b, :, :, 1] = _from_layout(of[:, 32:64])
        res_b[b, :, :, 0] = _from_layout(ob[:, 0:32])
        res_b[b, :, :, 1] = _from_layout(ob[:, 32:64])

    # forward-backward consistency (host; mirrors reference in fp32)
    counter = _bilinear_map_np(res_b, res_f)
    diff = np.max(np.abs(res_f - counter), axis=-1)
    invalid = (diff > EPS)[..., None]
    mf_t = matching_f.transpose(0, 2, 3, 1)  # [B,H,W,2]
    out = np.where(invalid, mf_t, res_f)
    return np.ascontiguousarray(out.transpose(0, 3, 1, 2)).astype(np.float32)


# revision 15
# speedup vs baseline: 1.1273x; 1.0778x over previous
"""PatchMatch-style MatchingPropagator on 8 Trainium2 NeuronCores.

Full inputs in, full outputs out. Sharding: 8 independent units =
(direction in {forward, backward}) x (batch 0..3), one NeuronCore each.

Key layout decisions:
- The host re-packs each unit's correlation volume into "quad" records
  Q[n, y0, x0, 0:4] = corr[n, y0:y0+2, x0:x0+2] for anchors in [0,62]^2,
  so every bilinear sample is ONE contiguous 16-byte indirect-DMA fetch.
  Clamping floors to <=62 is numerically identical to the reference's
  corner clamping.
- Every DVE op on the critical path reads/writes contiguous (or at most
  3-dim strided) access patterns; measured on TRN2, deep strided/broadcast
  views cost 2-3x a contiguous op of the same size.
- Candidate coords live in CC = [x-cols | y-cols] so floor/clamp/index
  ops are single wide contiguous ops; the [x|y|s] accept blocks in CT are
  filled by copies hidden under the gather's DMA flight time.
- The score uses prebuilt interleaved weight tiles UW = [u w u w] and
  TW = [t t wy wy] per pixel (built off the critical path), so the score
  is 2 contiguous multiplies + 3 stride-4 adds, bit-exact against the
  reference's product/sum order: s = ((t1+t2)+t3)+t4.
- The initial score eval is folded into the first propagate's gather
  (candidates pre-rolled on the host): 7 gathers total.

Pixel layout on chip: pixel (i, j) -> partition 64*(j//32) + i, free j%32.
"""

import numpy as np

B, H, W = 4, 64, 64
R = 3.0
EPS = np.float32(0.01)
N_CORES = 8
PIX = H * W              # 4096 pixels per unit
AN = W - 1               # 63 anchors per axis in the quad layout
QROW = AN * 4            # 252 floats per anchor row
QMAP = AN * AN * 4       # 15876 floats per pixel quad map
M_RNE = float(1 << 23)

_CACHE = {}


# ----------------------------------------------------------------------------
# Device program (SPMD: identical on all 8 cores; data differs per core)
# ----------------------------------------------------------------------------

def _build_program():
    import concourse.bass as bass
    import concourse.mybir as mybir
    import concourse.tile as tile
    from concourse import bacc

    F32 = mybir.dt.float32
    I32 = mybir.dt.int32
    OP = mybir.AluOpType
    AF = mybir.ActivationFunctionType

    nc = bacc.Bacc(
        "TRN2",
        target_bir_lowering=False,
        debug=False,
        enable_asserts=False,
        num_devices=N_CORES,
    )

    corr = nc.dram_tensor("corr", [PIX * QMAP], F32, kind="ExternalInput")
    # state cols (32 each): [x, y, hx1, hy1, vx1, vy1, base, nx1, ny1,
    #                        nx2, ny2, nx3, ny3]
    state_in = nc.dram_tensor("state", [128, 13 * 32], F32,
                              kind="ExternalInput")
    out_xy = nc.dram_tensor("out_xy", [128, 64], F32,
                            kind="ExternalOutput")

    corr_flat = corr.ap().rearrange("(n one) -> n one", one=1)

    def b3(ap):  # [128,32] -> broadcast [128,3,32]
        return ap.rearrange("p (one f) -> p one f", one=1).to_broadcast(
            [128, 3, 32])

    with tile.TileContext(nc) as tc:
        with tc.tile_pool(name="main", bufs=1) as pool:
            ST = pool.tile([128, 13 * 32], F32, name="ST")
            nc.sync.dma_start(ST[0:64], state_in.ap()[0:64])
            nc.scalar.dma_start(ST[64:128], state_in.ap()[64:128])
            BASE = ST[:, 192:224]

            def noise_view(k):
                o = 224 + 64 * k
                return ST[:, o:o + 64]  # [nx|ny]

            # CT accept blocks of 96: [BEST | H | V], each [x|y|s]
            CT = pool.tile([128, 288], F32, name="CT")
            # candidate coords as [x y] pairs; separate tiles so the
            # v row-roll DMA never serializes against h-chain DVE writes
            CCH = pool.tile([128, 64], F32, name="CCH")
            CCV = pool.tile([128, 64], F32, name="CCV")
            G = pool.tile([128, 768], F32, name="G")
            UW = pool.tile([128, 384], F32, name="UW")   # [u w u w] per px
            TW = pool.tile([128, 384], F32, name="TW")   # [t t wy wy] per px
            WT = pool.tile([128, 192], F32, name="WT")   # [w | wy] per slot
            XI = pool.tile([128, 192], I32, name="XI")
            IF = pool.tile([128, 96], I32, name="IF")
            I = pool.tile([128, 96], I32, name="I")
            B1 = pool.tile([128, 384], F32, name="B1")
            B2 = pool.tile([128, 384], F32, name="B2")
            UPD = pool.tile([128, 128], I32, name="UPD")
            RCS = pool.tile([128, 192], F32, name="RCS")  # 3-variant RC
            XIS = pool.tile([128, 192], I32, name="XIS")
            ISF = pool.tile([128, 96], I32, name="ISF")
            IS = pool.tile([128, 96], I32, name="IS")
            BASEI = pool.tile([128, 32], I32, name="BASEI")
            v0 = nc.vector
            v0.memset(I[:, 0:32], 0)
            nc.gpsimd.indirect_dma_start(
                out=G[:, 0:128],
                out_offset=None,
                in_=corr_flat,
                in_offset=bass.IndirectOffsetOnAxis(ap=I[:, 0:32], axis=0),
            )
            v0.tensor_copy(BASEI[:], ST[:, 192:224])

            v = nc.vector

            def eval_pre(cv, ne, off):
                """floor + clamp + quad indices for an eval slot of `ne`
                candidates whose [x y]-pair coords are the contiguous view
                cv.  Floors via truncating f32->i32 cast (coords >= 0),
                clamped <= 61+1 in int.  Slot regions start at 32-col
                block `off`."""
                n = 64 * ne
                m = 32 * ne
                x0 = XI[:, 2 * 32 * off:2 * 32 * off + n]
                v.tensor_scalar(x0, cv, float(AN - 1), None, OP.min)
                x2 = x0.rearrange("p (c s q) -> p c s q", c=ne, s=2)
                if3 = IF[:, 32 * off:32 * off + m].rearrange(
                    "p (e q) -> p e q", e=ne)
                i3 = I[:, 32 * off:32 * off + m].rearrange(
                    "p (e q) -> p e q", e=ne)
                baseb = (BASEI.rearrange("p (one f) -> p one f", one=1)
                         .to_broadcast([128, ne, 32]))
                v.scalar_tensor_tensor(if3, x2[:, :, 1], QROW, baseb,
                                       OP.mult, OP.add)
                v.scalar_tensor_tensor(i3, x2[:, :, 0], 4, if3,
                                       OP.mult, OP.add)

            def eval_gather(ne, off):
                nc.gpsimd.indirect_dma_start(
                    out=G[:, 128 * off:128 * (off + ne)],
                    out_offset=None,
                    in_=corr_flat,
                    in_offset=bass.IndirectOffsetOnAxis(
                        ap=I[:, 32 * off:32 * (off + ne)], axis=0),
                )

            def eval_weights(cv, ne, off):
                """hidden under gather flight: build UW = [u w u w] and
                TW = [t t wy wy] per pixel (all on DVE; no act tables)."""
                m = 32 * ne
                c2 = cv.rearrange("p (c s q) -> p c s q", c=ne, s=2)
                x2 = (XI[:, 64 * off:64 * off + 2 * m]
                      .rearrange("p (c s q) -> p c s q", c=ne, s=2))
                wcol = WT[:, 64 * off:64 * off + m]
                wycol = WT[:, 64 * off + m:64 * off + 2 * m]
                w = wcol.rearrange("p (e q) -> p e q", e=ne)
                wy = wycol.rearrange("p (e q) -> p e q", e=ne)
                v.tensor_tensor(w, c2[:, :, 0], x2[:, :, 0], OP.subtract)
                v.tensor_tensor(wy, c2[:, :, 1], x2[:, :, 1], OP.subtract)
                uwv = UW[:, 128 * off:128 * (off + ne)].rearrange(
                    "p (e d s) -> p e d s", e=m, d=2, s=2)
                twv = TW[:, 128 * off:128 * (off + ne)].rearrange(
                    "p (e s d) -> p e s d", e=m, s=2, d=2)
                wb = (wcol.rearrange("p (e one) -> p e one", one=1)
                      .to_broadcast([128, m, 2]))
                wyb = (wycol.rearrange("p (e one) -> p e one", one=1)
                       .to_broadcast([128, m, 2]))
                v.tensor_copy(uwv[:, :, :, 1], wb)
                v.tensor_copy(twv[:, :, 1, :], wyb)
                v.tensor_scalar(uwv[:, :, :, 0], wb, -1.0, 1.0,
                                OP.mult, OP.add)
                v.tensor_scalar(twv[:, :, 0, :], wyb, -1.0, 1.0,
                                OP.mult, OP.add)

            def eval_score(ne, off, sc_dst):
                """bilinear score; bit-exact term/sum order of the
                reference: t_k = (corner*u_or_w)*t_or_wy,
                s = ((t1+t2)+t3)+t4 via a sequential innermost reduce."""
                lo, hi = 128 * off, 128 * (off + ne)
                v.tensor_tensor(B1[:, lo:hi], G[:, lo:hi], UW[:, lo:hi],
                                OP.mult)
                v.tensor_tensor(B2[:, lo:hi], B1[:, lo:hi], TW[:, lo:hi],
                                OP.mult)
                b4 = B2[:, lo:hi].rearrange("p (e k) -> p e k", k=4)
                v.tensor_reduce(sc_dst, b4, mybir.AxisListType.X, OP.add)

            def accept(blk, last=False):
                """BEST = candidate block blk where its score is higher.
                The decision mask is kept in UPD slot blk-1 so a following
                speculative random-search can select by it."""
                so = 96 * blk
                mo = 64 * (blk - 1)

                def b2(ap):
                    return (ap.rearrange("p (one f) -> p one f", one=1)
                            .to_broadcast([128, 2, 32]))

                v.tensor_tensor(UPD[:, mo:mo + 64].rearrange(
                    "p (c f) -> p c f", c=2), b2(CT[:, so + 64:so + 96]),
                    b2(CT[:, 64:96]), OP.is_gt)
                v.copy_predicated(CT[:, 0:64], UPD[:, mo:mo + 64],
                                  CT[:, so:so + 64])
                if not last:
                    v.copy_predicated(CT[:, 64:96], UPD[:, mo:mo + 32],
                                      CT[:, so + 64:so + 96])

            def spec_rs(k):
                """Speculative random-search index precompute, hidden under
                the current gather's flight: candidate coords + quad
                indices for each possible accept outcome (B, H, V).
                The base variant lands directly in I/CCH; the accepts'
                masks later select the H/V variants."""
                cv3 = (CT[:].rearrange("p (b f) -> p b f", b=3)[:, :, 0:64])
                nzb = (noise_view(k)
                       .rearrange("p (one f) -> p one f", one=1)
                       .to_broadcast([128, 3, 64]))
                rc3 = RCS[:].rearrange("p (c f) -> p c f", c=3)
                v.tensor_tensor(rc3, cv3, nzb, OP.add)
                v.tensor_scalar(RCS[:], RCS[:], 0.0, float(W - 1),
                                OP.max, OP.min)
                v.tensor_scalar(XIS[:], RCS[:], float(AN - 1), None, OP.min)
                x2 = XIS[:].rearrange("p (c s q) -> p c s q", c=3, s=2)
                if3 = ISF[:].rearrange("p (e q) -> p e q", e=3)
                i3 = IS[:].rearrange("p (e q) -> p e q", e=3)
                baseb = (BASEI.rearrange("p (one f) -> p one f", one=1)
                         .to_broadcast([128, 3, 32]))
                v.scalar_tensor_tensor(if3, x2[:, :, 1], QROW, baseb,
                                       OP.mult, OP.add)
                v.scalar_tensor_tensor(i3, x2[:, :, 0], 4, if3,
                                       OP.mult, OP.add)
                nc.gpsimd.indirect_dma_start(
                    out=G[:, 384:768],
                    out_offset=None,
                    in_=corr_flat,
                    in_offset=bass.IndirectOffsetOnAxis(ap=IS[:], axis=0),
                )
                v.tensor_copy(CCH[:], RCS[:, 0:64])

            def sc_block(blk, nb=1):
                """CT score-column view [128, nb, 32] from block blk."""
                return (CT[:].rearrange("p (b f) -> p b f", b=3)
                        [:, blk:blk + nb, 64:96])

            # ---- round 1: initial eval + propagate(1,1); candidate coords
            # pre-rolled on the host.  Split into a (best,h) chain and a v
            # chain so the second gather's descriptor gen overlaps the
            # first's flight.
            eval_pre(ST[:, 0:192], 3, 0)
            eval_gather(3, 0)
            eval_weights(ST[:, 0:192], 3, 0)
            v.tensor_copy(CT[:, 0:64], ST[:, 0:64])
            v.tensor_copy(CT[:, 96:160], ST[:, 64:128])
            v.tensor_copy(CT[:, 192:256], ST[:, 128:192])
            spec_rs(0)
            eval_score(3, 0, sc_block(0, 3))
            accept(1)
            accept(2)

            def propagate(dx, dy, spec_k=None, last=False):
                # cand_v coords: row-roll of BEST [x|y] by dy via 2 fused-AP
                # DMAs (bulk + wrap) on the two HWDGE issuers; issued first
                # so the DMA latency overlaps the whole h-chain prep
                dvv = CCV[:].rearrange("(b i) f -> b i f", b=2)
                svv = CT[:, 0:64].rearrange("(b i) f -> b i f", b=2)
                if dy == 1:
                    nc.sync.dma_start(dvv[0:1, 1:64], svv[0:1, 0:63])
                    nc.scalar.dma_start(dvv[1:2, 1:64], svv[1:2, 0:63])
                    nc.sync.dma_start(dvv[1:2, 0:1], svv[1:2, 63:64])
                    nc.scalar.dma_start(dvv[0:1, 0:1], svv[0:1, 63:64])
                else:
                    nc.sync.dma_start(dvv[0:1, 0:63], svv[0:1, 1:64])
                    nc.scalar.dma_start(dvv[1:2, 0:63], svv[1:2, 1:64])
                    nc.sync.dma_start(dvv[1:2, 63:64], svv[1:2, 0:1])
                    nc.scalar.dma_start(dvv[0:1, 63:64], svv[0:1, 0:1])

                # ---- h chain: col-roll by dx (DVE), clamp, idx; its DVE
                # work and weight prep hide the v row-roll DMA latency
                dh = CCH[:].rearrange("p (c f) -> p c f", c=2)
                sh = CT[:, 0:64].rearrange("p (c f) -> p c f", c=2)
                if dx == 1:
                    v.tensor_copy(dh[:, :, 1:32], sh[:, :, 0:31])
                    v.tensor_copy(dh[64:128, :, 0:1], sh[0:64, :, 31:32])
                    v.tensor_copy(dh[0:64, :, 0:1], sh[64:128, :, 31:32])
                    v.tensor_scalar(CCH[:, 0:32], CCH[:, 0:32], 1.0,
                                    float(W - 1), OP.add, OP.min)
                else:
                    v.tensor_copy(dh[:, :, 0:31], sh[:, :, 1:32])
                    v.tensor_copy(dh[0:64, :, 31:32], sh[64:128, :, 0:1])
                    v.tensor_copy(dh[64:128, :, 31:32], sh[0:64, :, 0:1])
                    v.tensor_scalar(CCH[:, 0:32], CCH[:, 0:32], -1.0, 0.0,
                                    OP.add, OP.max)
                eval_pre(CCH[:], 1, 0)
                eval_weights(CCH[:], 1, 0)
                v.tensor_copy(CT[:, 96:160], CCH[:])

                # ---- v chain (waits the roll DMA), then one fused gather
                if dy == 1:
                    v.tensor_scalar(CCV[:, 32:64], CCV[:, 32:64], 1.0,
                                    float(H - 1), OP.add, OP.min)
                else:
                    v.tensor_scalar(CCV[:, 32:64], CCV[:, 32:64], -1.0,
                                    0.0, OP.add, OP.max)
                eval_pre(CCV[:], 1, 1)
                eval_gather(2, 0)

                # hidden work under the gather flight
                eval_weights(CCV[:], 1, 1)
                v.tensor_copy(CT[:, 192:256], CCV[:])
                if spec_k is not None:
                    spec_rs(spec_k)
                eval_score(2, 0, sc_block(1, 2))
                accept(1)
                accept(2, last=last)

            def random_search(k):
                # corner data for all 3 variants was gathered during the
                # propagate's flight; select values and coords by the
                # accept masks, then score directly - no DMA on this path
                def m4(mo):
                    return (UPD[:, mo:mo + 32]
                            .rearrange("p (q one) -> p q one", one=1)
                            .to_broadcast([128, 32, 4]))

                g4 = G[:, 384:512].rearrange("p (q k) -> p q k", k=4)
                gh = G[:, 512:640].rearrange("p (q k) -> p q k", k=4)
                gv = G[:, 640:768].rearrange("p (q k) -> p q k", k=4)
                v.copy_predicated(g4, m4(0), gh)
                v.copy_predicated(g4, m4(64), gv)
                v.copy_predicated(CCH[:], UPD[:, 0:64], RCS[:, 64:128])
                v.copy_predicated(CCH[:], UPD[:, 64:128], RCS[:, 128:192])
                v.tensor_scalar(XI[:, 0:64], CCH[:], float(AN - 1),
                                None, OP.min)
                eval_weights(CCH[:], 1, 0)
                v.tensor_copy(CT[:, 96:160], CCH[:])
                v.tensor_tensor(B1[:, 0:128], G[:, 384:512], UW[:, 0:128],
                                OP.mult)
                v.tensor_tensor(B2[:, 0:128], B1[:, 0:128], TW[:, 0:128],
                                OP.mult)
                b4 = B2[:, 0:128].rearrange("p (e k) -> p e k", k=4)
                v.tensor_reduce(sc_block(1), b4, mybir.AxisListType.X,
                                OP.add)
                accept(1)

            random_search(0)
            propagate(-1, -1, spec_k=1)
            random_search(1)
            propagate(-1, 1, spec_k=2)
            random_search(2)
            propagate(1, -1, last=True)

            nc.sync.dma_start(out_xy.ap(), CT[:, 0:64])

    nc.compile()
    return nc


def _get_program():
    if "nc" not in _CACHE:
        _CACHE["nc"] = _build_program()
    return _CACHE["nc"]


# ----------------------------------------------------------------------------
# Host-side helpers
# ----------------------------------------------------------------------------

def _to_layout(v):
    """[64(i), 64(j)] -> [128, 32]; partition = 64*(j//32)+i, free = j%32."""
    return np.ascontiguousarray(
        v.reshape(64, 2, 32).transpose(1, 0, 2).reshape(128, 32))


def _from_layout(a):
    """[128, 32] -> [64(i), 64(j)]."""
    return a.reshape(2, 64, 32).transpose(1, 0, 2).reshape(64, 64)


def _noise_arrays():
    """Mirror the reference's jax.random usage exactly, in-process, so the
    values match the grader's reference no matter which jax backend/PRNG
    the process defaults to."""
    import jax
    import jax.numpy as jnp

    key = jax.random.key(42)
    kf, kb = jax.random.split(key)
    out = []
    for kdir in (kf, kb):
        ks = jax.random.split(kdir, 3)
        out.append([np.asarray(R * jax.random.normal(k, (B, H, W, 2),
                                                     jnp.float32))
                    for k in ks])
    return out  # [dir][step] -> [B,H,W,2] float32


def _quad_pack(corr_u):
    """[4096, 64, 64] -> flat quad records [4096*63*63*4] f32."""
    sw = np.lib.stride_tricks.sliding_window_view(corr_u, (2, 2),
                                                  axis=(1, 2))
    # sw: [4096, 63, 63, 2, 2]
    return np.ascontiguousarray(sw).reshape(-1)


def _make_state(x_plane, y_plane, noise_steps, b):
    """Build the [128, 13*32] per-core state tensor (partition-major)."""
    x = x_plane.astype(np.float32)
    y = y_plane.astype(np.float32)
    one = np.float32(1.0)
    # first propagate is (dx, dy) = (1, 1); host pre-rolls the candidates
    hx = np.clip(np.roll(x, 1, axis=1) + one, np.float32(0.0),
                 np.float32(W - 1))
    hy = np.roll(y, 1, axis=1)
    vx = np.roll(x, 1, axis=0)
    vy = np.clip(np.roll(y, 1, axis=0) + one, np.float32(0.0),
                 np.float32(H - 1))
    base = ((np.arange(64, dtype=np.int64)[:, None] * 64
             + np.arange(64, dtype=np.int64)[None, :]) * QMAP)
    rows = [
        _to_layout(x), _to_layout(y),
        _to_layout(hx), _to_layout(hy),
        _to_layout(vx), _to_layout(vy),
        _to_layout(base.astype(np.float32)),
    ]
    for step in range(3):
        nz = noise_steps[step][b]  # [H,W,2]
        rows.append(_to_layout(np.ascontiguousarray(nz[:, :, 0])))
        rows.append(_to_layout(np.ascontiguousarray(nz[:, :, 1])))
    return np.concatenate(rows, axis=1).astype(np.float32)


def _bilinear_map_np(img, coords):
    """numpy mirror of reference._bilinear_map (fp32, same op order).
    img [B,H,W,C], coords [B,H,W,2] -> [B,H,W,C]"""
    Bn, Hn, Wn, C = img.shape
    out = np.empty_like(img)
    one = np.float32(1.0)
    for b in range(Bn):
        x = coords[b, :, :, 0].reshape(-1)
        y = coords[b, :, :, 1].reshape(-1)
        x0 = np.floor(x)
        y0 = np.floor(y)
        wx = (x - x0)[:, None]
        wy = (y - y0)[:, None]
        x0i = np.clip(x0.astype(np.int32), 0, Wn - 1)
        x1i = np.clip(x0i + 1, 0, Wn - 1)
        y0i = np.clip(y0.astype(np.int32), 0, Hn - 1)
        y1i = np.clip(y0i + 1, 0, Hn - 1)
        im = img[b]
        v00 = im[y0i, x0i]
        v01 = im[y0i, x1i]
        v10 = im[y1i, x0i]
        v11 = im[y1i, x1i]
        o = (v00 * (one - wx) * (one - wy) + v01 * wx * (one - wy)
             + v10 * (one - wx) * wy + v11 * wx * wy)
        out[b] = o.reshape(Hn, Wn, C)
    return out


def _run_device(in_maps, trace=False):
    from concourse import bass_utils

    nc = _get_program()
    res = bass_utils.run_bass_kernel_spmd(
        nc, in_maps, core_ids=list(range(N_CORES)), trace=trace)
    return res


def kernel(matching_f, matching_b, corr_map, _trace=False, _results_hook=None):
    matching_f = np.asarray(matching_f)
    matching_b = np.asarray(matching_b)
    corr_map = np.asarray(corr_map)

    noise = _noise_arrays()  # [dir][step][B,H,W,2]

    in_maps = []
    for b in range(B):  # forward units, cores 0..3
        corr_u = np.ascontiguousarray(corr_map[b]).reshape(PIX, H, W)
        in_maps.append({
            "corr": _quad_pack(corr_u),
            "state": _make_state(matching_f[b, 0], matching_f[b, 1],
                                 noise[0], b),
        })
    for b in range(B):  # backward units, cores 4..7
        corr_t = np.ascontiguousarray(
            corr_map[b].transpose(2, 3, 0, 1)).reshape(PIX, H, W)
        in_maps.append({
            "corr": _quad_pack(corr_t),
            "state": _make_state(matching_b[b, 0], matching_b[b, 1],
                                 noise[1], b),
        })

    res = _run_device(in_maps, trace=_trace)
    if _results_hook is not None:
        _results_hook(res)

    res_f = np.empty((B, H, W, 2), np.float32)
    res_b = np.empty((B, H, W, 2), np.float32)
    for b in range(B):
        of = res.results[b]["out_xy"]
        ob = res.results[4 + b]["out_xy"]
        res_f[b, :, :, 0] = _from_layout(of[:, 0:32])
        res_f[b, :, :, 1] = _from_layout(of[:, 32:64])
        res_b[b, :, :, 0] = _from_layout(ob[:, 0:32])
        res_b[b, :, :, 1] = _from_layout(ob[:, 32:64])

    # forward-backward consistency (host; mirrors reference in fp32)
    counter = _bilinear_map_np(res_b, res_f)
    diff = np.max(np.abs(res_f - counter), axis=-1)
    invalid = (diff > EPS)[..., None]
    mf_t = matching_f.transpose(0, 2, 3, 1)  # [B,H,W,2]
    out = np.where(invalid, mf_t, res_f)
    return np.ascontiguousarray(out.transpose(0, 3, 1, 2)).astype(np.float32)
